# revision 18
# baseline (speedup 1.0000x reference)
"""MoE FFN (8 experts, top-2) on 8 TRN2 NeuronCores, expert-parallel.

Strategy:
  - Host: router (fp64 logits -> softmax -> top-2 -> renormalized combine
    weights), gather each expert's assigned tokens, pad to a common
    capacity C (SPMD: one program, per-core inputs).
  - Core e: full SwiGLU FFN for expert e over its C tokens in bf16
    (full PE rate, half the DMA of fp32r), combine-weight scaling on
    device; outputs y^T [1024, C] bf16 (host transposes on unshard).
  - Host: scatter-add per-expert outputs back into [B, S, D].

Device kernel structure (single pass over all C tokens):
  Phase 1 (gate/up): x kept fully in SBUF; sweeps of <=3 token groups
    (512-col PSUM banks, 128-aligned widths), k-outer matmul ordering so
    one stationary (weight k-tile) feeds all groups of a sweep; SwiGLU
    into a resident bf16 hbuf [128,32,C].
    (Measured on HW: per-matmul instruction issue and ldweights reloads
    are ~free; >512-col matmuls are rejected by walrus
    (s3d3_mm_num_elements), so 512-col groups are already optimal.)
  Phase 2 (down): SWAPPED operands — stationary = down_w tile
    [128h, 128dout], moving = hbuf token columns (4 matmuls x 512 per
    stationary). Amortizes PE weight loads 4x and streams down_w once
    (not once per token chunk). Accumulate over 32 h-tiles into 4 PSUM
    banks per dout tile (2 dout tiles in flight = 8 banks). Output is
    y^T [dout, tokens]; combine-weight scale on evac via a
    partition-replicated cw row.

  Post-compile pass: delete InstLdweights whose weights AP equals the
  immediately preceding ldweights (PE weight registers persist across
  matmuls) — the tile pipeline emits one ldweights per matmul even for
  back-to-back matmuls sharing a stationary.

Layouts (host-prepared, DMA-friendly):
  xT   [8, 128, C]      x[idx].T split along d into 8 k-tiles (bf16)
  gw/uw[32, 128, 8, 128] gate/up ^T tiled: [h_tile][d_sub][k][h] (bf16)
  dw   [8, 128, 4096]   down^T tiled:     [dout_tile][h_sub][hi*128+dout] (bf16)
  cwF  [128, C]         combine weights, replicated across partitions (f32)
"""
import sys, os
for p in ("/opt/trn_rl_repo", os.path.join(os.path.dirname(os.path.abspath(__file__)))):
    if p not in sys.path:
        sys.path.insert(0, p)
import numpy as np
import ml_dtypes

BF16 = ml_dtypes.bfloat16
D_MODEL = 1024
D_INNER = 4096
N_EXPERTS = 8
TOP_K = 2
H_TILES = D_INNER // 128  # 32
K_TILES = D_MODEL // 128  # 8


CAPACITY = 2048  # capacity-factor-1.0: perfect expert balance on device;
                 # the few overflow pairs are combined on the host in f32.


def _capacity(max_n: int) -> int:
    return max(256, min(CAPACITY, ((max_n + 127) // 128) * 128))


def _p1_groups(C: int):
    """Token groups for phase 1: <=512 wide (one PSUM bank), 128-aligned.
    Full 512-wide groups plus one tail measured fastest on HW."""
    return [512] * (C // 512) + ([C % 512] if C % 512 else [])


def _dedupe_ldweights(nc) -> int:
    """Drop InstLdweights that reload the exact stationary already in the
    PE array (identical weights AP as the immediately preceding ldweights,
    with no intervening weight-modifying instruction, and no sync of its
    own). PE weight registers persist across matmuls, so the reload is
    pure overhead on the PE queue."""
    removed = 0
    for b in nc.m.functions[0].blocks:
        insts = b.instructions
        last_sig = None
        drop = []
        for idx, i in enumerate(insts):
            tn = type(i).__name__
            if tn == "InstLdweights":
                ap = i.ins[0]
                sig = (
                    getattr(ap, "memref", None), ap.offset, str(ap.ap),
                    i.perf_mode, i.is_transpose, i.tile_position, i.tile_size,
                )
                si = i.sync_info
                has_sync = si is not None and (
                    len(si.on_wait) > 0 or len(si.on_update) > 0)
                if sig == last_sig and not has_sync:
                    drop.append(idx)
                    continue
                last_sig = sig
            elif tn == "InstMatmult":
                continue  # weights unchanged; keep last_sig live
            elif tn in ("InstEventSemaphore", "InstDMACopy"):
                continue  # no effect on PE weight registers
            else:
                last_sig = None  # conservative across anything else
        for idx in reversed(drop):
            del insts[idx]
        removed += len(drop)
    return removed


def _build_nc(C: int, reps: int = 1, fast_start: bool = True, polish: bool = False, tail128: bool = False, dedupe: bool = True, p1wide: bool = False, sw: int = 3, wide: bool = False):
    import concourse.bass as bass
    import concourse.mybir as mybir
    import concourse.tile as tile
    from concourse import bacc
    from contextlib import nullcontext

    f32 = mybir.dt.float32
    bf16 = mybir.dt.bfloat16
    Silu = mybir.ActivationFunctionType.Silu

    assert C % 128 == 0
    groups = _p1_groups(C)
    assert sum(groups) == C and all(g <= 512 for g in groups)
    sweeps = [groups[i:i + sw] for i in range(0, len(groups), sw)]
    # phase-2 token chunks: 512-wide PSUM banks, dout-tile stationary
    q_chunks = [512] * (C // 512) + ([C % 512] if C % 512 else [])

    nc = bacc.Bacc(None, target_bir_lowering=False)
    xT_d = nc.dram_tensor("xT", [K_TILES, 128, C], bf16, kind="ExternalInput")
    gw_d = nc.dram_tensor("gw", [H_TILES, 128, K_TILES, 128], bf16, kind="ExternalInput")
    uw_d = nc.dram_tensor("uw", [H_TILES, 128, K_TILES, 128], bf16, kind="ExternalInput")
    dw_d = nc.dram_tensor("dw", [K_TILES, 128, H_TILES * 128], bf16, kind="ExternalInput")
    cw_d = nc.dram_tensor("cwF", [128, C], f32, kind="ExternalInput")
    y_d = nc.dram_tensor("y", [D_MODEL, C], bf16, kind="ExternalOutput")

    with tile.TileContext(nc) as tc:
        with (
            tc.tile_pool(name="xt", bufs=1) as xt_pool,
            tc.tile_pool(name="wgt", bufs=6 if polish else 4) as wgt_pool,
            tc.tile_pool(name="dwp", bufs=3) as dw_pool,
            tc.tile_pool(name="hb", bufs=1) as hb_pool,
            tc.tile_pool(name="sg", bufs=2 if p1wide else 3) as sg_pool,
            tc.tile_pool(name="yo", bufs=2 if p1wide else 4) as y_pool,
            tc.tile_pool(name="cw", bufs=1) as cw_pool,
            tc.tile_pool(name="ps", bufs=8, space="PSUM") as ps,
        ):
            cw_sb = cw_pool.tile([128, C], f32)
            nc.sync.dma_start(cw_sb[:], cw_d[:])

            rep_ctx = (
                tc.For_i(0, reps, 1,
                         hint_engines=(mybir.EngineType.PE, mybir.EngineType.SP))
                if reps > 1 else nullcontext()
            )
            with rep_ctx:
                hoisted = []
                if fast_start:
                    # hoist leading h-tiles' weight DMAs ahead of the bulk x
                    # transfer, and give each k-slice of x its own tile so
                    # the first matmuls gate on single-k DMAs
                    nhoist = 4 if polish else 1
                    for h in range(nhoist):
                        gwh = wgt_pool.tile([128, K_TILES, 128], bf16, tag="gw", name=f"gw{h}")
                        nc.sync.dma_start(gwh[:], gw_d[h])
                        uwh = wgt_pool.tile([128, K_TILES, 128], bf16, tag="uw", name=f"uw{h}")
                        nc.sync.dma_start(uwh[:], uw_d[h])
                        hoisted.append((gwh, uwh))
                    splitc = sum(sweeps[0]) if (polish and len(sweeps) > 1) else C
                    xtkA = [xt_pool.tile([128, splitc], bf16, tag=f"xtka{k}", name=f"xtka{k}")
                            for k in range(K_TILES)]
                    for k in range(K_TILES):
                        nc.sync.dma_start(xtkA[k][:], xT_d[k][:, 0:splitc])
                    xtkB = None
                    if splitc < C:
                        xtkB = [xt_pool.tile([128, C - splitc], bf16, tag=f"xtkb{k}", name=f"xtkb{k}")
                                for k in range(K_TILES)]
                        for k in range(K_TILES):
                            nc.sync.dma_start(xtkB[k][:], xT_d[k][:, splitc:])
                    def xs(k, gs, gsz):
                        if gs >= splitc:
                            return xtkB[k][:, gs - splitc:gs - splitc + gsz]
                        return xtkA[k][:, gs:gs + gsz]
                else:
                    xt = xt_pool.tile([128, K_TILES, C], bf16, tag="xt")
                    for k in range(K_TILES):
                        nc.sync.dma_start(xt[:, k, :], xT_d[k])
                    def xs(k, gs, gsz):
                        return xt[:, k, gs:gs + gsz]
                hbuf = hb_pool.tile([128, H_TILES, C], bf16, tag="hbuf")

                # ---- phase 1: gate/up + SwiGLU into hbuf
                if wide:
                    # one multi-bank (4x512) matmul per (hi, k): moving spans
                    # all C tokens; minimizes PE instruction count.
                    assert not polish
                    gl = []
                    g0 = 0
                    for gsz in groups:
                        gl.append((g0, gsz))
                        g0 += gsz
                    for hi in range(H_TILES):
                        if hoisted:
                            gw, uw = hoisted.pop(0)
                        else:
                            gw = wgt_pool.tile([128, K_TILES, 128], bf16, tag="gw")
                            nc.sync.dma_start(gw[:], gw_d[hi])
                            uw = wgt_pool.tile([128, K_TILES, 128], bf16, tag="uw")
                            nc.sync.dma_start(uw[:], uw_d[hi])
                        pg = ps.tile([128, C], f32, tag="ps", name="pg")
                        for k in range(K_TILES):
                            nc.tensor.matmul(
                                pg[:], gw[:, k, :], xs(k, 0, C),
                                start=(k == 0), stop=(k == K_TILES - 1))
                        sgb = sg_pool.tile([128, C], bf16, tag="sg", name="sgb")
                        for (gs, gsz) in gl:
                            nc.scalar.activation(sgb[:, gs:gs + gsz], pg[:, gs:gs + gsz], Silu)
                        pu = ps.tile([128, C], f32, tag="ps", name="pu")
                        for k in range(K_TILES):
                            nc.tensor.matmul(
                                pu[:], uw[:, k, :], xs(k, 0, C),
                                start=(k == 0), stop=(k == K_TILES - 1))
                        for (gs, gsz) in gl:
                            nc.vector.tensor_mul(
                                hbuf[:, hi, gs:gs + gsz], sgb[:, gs:gs + gsz],
                                pu[:, gs:gs + gsz])
                    sweeps_eff = []
                elif p1wide:
                    # gate over ALL token groups per stationary (4 PSUM
                    # banks), silu into a bf16 staging row, then up over all
                    # groups (other 4 banks), mul into hbuf. Stationary runs
                    # of len(groups) amortize PE weight loads.
                    gl = []
                    g0 = 0
                    for gsz in groups:
                        gl.append((g0, gsz))
                        g0 += gsz
                    for hi in range(H_TILES):
                        if hoisted:
                            gw, uw = hoisted.pop(0)
                        else:
                            gw = wgt_pool.tile([128, K_TILES, 128], bf16, tag="gw")
                            nc.sync.dma_start(gw[:], gw_d[hi])
                            uw = wgt_pool.tile([128, K_TILES, 128], bf16, tag="uw")
                            nc.sync.dma_start(uw[:], uw_d[hi])
                        pg = [ps.tile([128, gsz], f32, tag="ps", name="pg",
                                      padded_shape=[128, 512]) for (_, gsz) in gl]
                        for k in range(K_TILES):
                            for j, (gs, gsz) in enumerate(gl):
                                nc.tensor.matmul(
                                    pg[j][:], gw[:, k, :], xs(k, gs, gsz),
                                    start=(k == 0), stop=(k == K_TILES - 1))
                        sgb = sg_pool.tile([128, C], bf16, tag="sg", name="sgb")
                        for j, (gs, gsz) in enumerate(gl):
                            nc.scalar.activation(sgb[:, gs:gs + gsz], pg[j][:], Silu)
                        pu = [ps.tile([128, gsz], f32, tag="ps", name="pu",
                                      padded_shape=[128, 512]) for (_, gsz) in gl]
                        for k in range(K_TILES):
                            for j, (gs, gsz) in enumerate(gl):
                                nc.tensor.matmul(
                                    pu[j][:], uw[:, k, :], xs(k, gs, gsz),
                                    start=(k == 0), stop=(k == K_TILES - 1))
                        for j, (gs, gsz) in enumerate(gl):
                            nc.vector.tensor_mul(
                                hbuf[:, hi, gs:gs + gsz], sgb[:, gs:gs + gsz], pu[j][:])
                    sweeps_eff = []
                else:
                    sweeps_eff = sweeps
                t0 = 0
                for sw in sweeps_eff:
                    sl = []
                    g0 = t0
                    for gsz in sw:
                        sl.append((g0, gsz))
                        g0 += gsz
                    for hi in range(H_TILES):
                        if hoisted:
                            gw, uw = hoisted.pop(0)
                        else:
                            gw = wgt_pool.tile([128, K_TILES, 128], bf16, tag="gw")
                            nc.sync.dma_start(gw[:], gw_d[hi])
                            uw = wgt_pool.tile([128, K_TILES, 128], bf16, tag="uw")
                            nc.sync.dma_start(uw[:], uw_d[hi])
                        pg = [ps.tile([128, gsz], f32, tag="ps", name="pg",
                                      padded_shape=[128, 512]) for (_, gsz) in sl]
                        pu = [ps.tile([128, gsz], f32, tag="ps", name="pu",
                                      padded_shape=[128, 512]) for (_, gsz) in sl]
                        for k in range(K_TILES):
                            for j, (gs, gsz) in enumerate(sl):
                                nc.tensor.matmul(
                                    pg[j][:], gw[:, k, :], xs(k, gs, gsz),
                                    start=(k == 0), stop=(k == K_TILES - 1))
                            for j, (gs, gsz) in enumerate(sl):
                                nc.tensor.matmul(
                                    pu[j][:], uw[:, k, :], xs(k, gs, gsz),
                                    start=(k == 0), stop=(k == K_TILES - 1))
                        for j, (gs, gsz) in enumerate(sl):
                            sg = sg_pool.tile([128, gsz], f32, tag="sg", name="sg",
                                              padded_shape=[128, 512])
                            nc.scalar.activation(sg[:], pg[j][:], Silu)
                            nc.vector.tensor_mul(hbuf[:, hi, gs:gs + gsz], sg[:], pu[j][:])
                    t0 = g0

                # ---- phase 2 (swapped): stationary = down_w tile
                # [128h, 128dout]; moving = hbuf token columns. Output is
                # y^T [dout, tokens]; scale by partition-replicated cw.
                HH = H_TILES // 2
                for dt in range(K_TILES):
                    dwt2 = []
                    for h2 in range(2):
                        dwh = dw_pool.tile([128, HH * 128], bf16, tag="dw", name="dwt")
                        nc.sync.dma_start(dwh[:], dw_d[dt][:, h2 * HH * 128:(h2 + 1) * HH * 128])
                        dwt2.append(dwh)
                    yp = [ps.tile([128, qs], f32, tag="ps", name="yp",
                                  padded_shape=[128, 512]) for qs in q_chunks]
                    for hi in range(H_TILES):
                        st = dwt2[hi // HH][:, (hi % HH) * 128:(hi % HH + 1) * 128]
                        q0 = 0
                        for q, qs in enumerate(q_chunks):
                            nc.tensor.matmul(
                                yp[q][:], st, hbuf[:, hi, q0:q0 + qs],
                                start=(hi == 0), stop=(hi == H_TILES - 1))
                            q0 += qs
                    q0 = 0
                    for q, qs in enumerate(q_chunks):
                        yt = y_pool.tile([128, qs], bf16, tag="yt", name="yt",
                                         padded_shape=[128, 512])
                        nc.vector.tensor_mul(yt[:], yp[q][:], cw_sb[:, q0:q0 + qs])
                        nc.sync.dma_start(
                            y_d[dt * 128:(dt + 1) * 128, q0:q0 + qs], yt[:])
                        q0 += qs
    nc.compile()
    if dedupe:
        n = _dedupe_ldweights(nc)
        import logging
        logging.getLogger(__name__).info(f"deduped {n} ldweights")
    import concourse.bass as _bass
    _bass.Bass.finalize(nc)
    return nc


_NC_CACHE: dict = {}


def _get_nc(C: int):
    if C not in _NC_CACHE:
        _NC_CACHE[C] = _build_nc(C)
    return _NC_CACHE[C]


def _route(x2d: np.ndarray, router_w: np.ndarray, router_b: np.ndarray):
    """fp64 router: returns (idx_per_expert, cw_per_expert) lists."""
    logits = x2d.astype(np.float64) @ router_w.astype(np.float64).T + router_b.astype(np.float64)
    m = logits.max(axis=-1, keepdims=True)
    p = np.exp(logits - m)
    p /= p.sum(axis=-1, keepdims=True)
    # top-2 (jax.lax.top_k picks largest; softmax is monotonic in logits)
    i1 = np.argmax(p, axis=-1)
    p_masked = p.copy()
    p_masked[np.arange(p.shape[0]), i1] = -1.0
    i2 = np.argmax(p_masked, axis=-1)
    p1 = p[np.arange(p.shape[0]), i1]
    p2 = p[np.arange(p.shape[0]), i2]
    denom = p1 + p2
    w1 = p1 / denom
    w2 = p2 / denom
    idxs, cws = [], []
    for e in range(N_EXPERTS):
        sel1 = np.nonzero(i1 == e)[0]
        sel2 = np.nonzero(i2 == e)[0]
        idx = np.concatenate([sel1, sel2])
        cw = np.concatenate([w1[sel1], w2[sel2]])
        idxs.append(idx)
        cws.append(cw.astype(np.float32))
    return idxs, cws


def _prep_core_inputs(x2d, idxs, cws, gate_w, up_w, down_w, C):
    in_maps = []
    for e in range(N_EXPERTS):
        idx = idxs[e]
        n = len(idx)
        xe = np.zeros((C, D_MODEL), np.float32)
        xe[:n] = x2d[idx]
        xT = np.ascontiguousarray(xe.T).astype(BF16).reshape(K_TILES, 128, C)
        gw = np.ascontiguousarray(
            gate_w[e].T.reshape(K_TILES, 128, H_TILES, 128).transpose(2, 1, 0, 3)).astype(BF16)
        uw = np.ascontiguousarray(
            up_w[e].T.reshape(K_TILES, 128, H_TILES, 128).transpose(2, 1, 0, 3)).astype(BF16)
        # [dout_tile, h_sub, hi*128+dout]: contiguous stationary tiles for
        # the swapped phase 2 (one 1MB DMA per dout tile)
        dw = np.ascontiguousarray(
            down_w[e].T.reshape(H_TILES, 128, K_TILES, 128)
            .transpose(2, 1, 0, 3).reshape(K_TILES, 128, H_TILES * 128)).astype(BF16)
        cw = np.zeros((C,), np.float32)
        cw[:n] = cws[e]
        cwF = np.ascontiguousarray(np.broadcast_to(cw[None, :], (128, C)))
        in_maps.append({"xT": xT, "gw": gw, "uw": uw, "dw": dw, "cwF": cwF})
    return in_maps


def _silu(v):
    return v / (1.0 + np.exp(-v))


def kernel(x, router_w, router_b, gate_w, up_w, down_w):
    from concourse.bass_utils import run_bass_kernel_spmd

    x = np.asarray(x, dtype=np.float32)
    router_w = np.asarray(router_w, dtype=np.float32)
    router_b = np.asarray(router_b, dtype=np.float32)
    gate_w = np.asarray(gate_w, dtype=np.float32)
    up_w = np.asarray(up_w, dtype=np.float32)
    down_w = np.asarray(down_w, dtype=np.float32)

    B, S, D = x.shape
    x2d = x.reshape(B * S, D)
    idxs, cws = _route(x2d, router_w, router_b)
    max_n = max(len(i) for i in idxs)
    C = _capacity(max_n)

    # device gets the first C pairs per expert; overflow handled on host
    dev_idxs = [i[:C] for i in idxs]
    dev_cws = [c[:C] for c in cws]

    nc = _get_nc(C)
    in_maps = _prep_core_inputs(x2d, dev_idxs, dev_cws, gate_w, up_w, down_w, C)
    res = run_bass_kernel_spmd(nc, in_maps, core_ids=list(range(N_EXPERTS)), trace=False)

    out = np.zeros((B * S, D_MODEL), np.float32)
    for e in range(N_EXPERTS):
        n = len(dev_idxs[e])
        ye = res.results[e]["y"].astype(np.float32).T  # [C, D_MODEL]
        np.add.at(out, dev_idxs[e], ye[:n])
        if len(idxs[e]) > C:  # capacity overflow: combine on host in f32
            oi = idxs[e][C:]
            ocw = cws[e][C:]
            xs = x2d[oi]
            h = _silu(xs @ gate_w[e].T) * (xs @ up_w[e].T)
            np.add.at(out, oi, ocw[:, None] * (h @ down_w[e].T))
    return out.reshape(B, S, D_MODEL)



# revision 25
# speedup vs baseline: 1.0279x; 1.0279x over previous
"""MoE FFN (8 experts, top-2) on 8 TRN2 NeuronCores, expert-parallel.

Strategy:
  - Host: router (fp64 logits -> softmax -> top-2 -> renormalized combine
    weights), gather each expert's assigned tokens, pad to a common
    capacity C (SPMD: one program, per-core inputs).
  - Core e: full SwiGLU FFN for expert e over its C tokens in bf16
    (full PE rate, half the DMA of fp32r), combine-weight scaling on
    device; outputs y^T [1024, C] bf16 (host transposes on unshard).
  - Host: scatter-add per-expert outputs back into [B, S, D].

Device kernel structure (single pass over all C tokens):
  Phase 1 (gate/up): x kept fully in SBUF; sweeps of <=3 token groups
    (512-col PSUM banks, 128-aligned widths), k-outer matmul ordering so
    one stationary (weight k-tile) feeds all groups of a sweep; SwiGLU
    into a resident bf16 hbuf [128,32,C].
    (Measured on HW: per-matmul instruction issue and ldweights reloads
    are ~free; >512-col matmuls are rejected by walrus
    (s3d3_mm_num_elements), so 512-col groups are already optimal.)
  Phase 2 (down): SWAPPED operands — stationary = down_w tile
    [128h, 128dout], moving = hbuf token columns (4 matmuls x 512 per
    stationary). Amortizes PE weight loads 4x and streams down_w once
    (not once per token chunk). Accumulate over 32 h-tiles into 4 PSUM
    banks per dout tile (2 dout tiles in flight = 8 banks). Output is
    y^T [dout, tokens]; combine-weight scale on evac via a
    partition-replicated cw row.

  Post-compile pass: delete InstLdweights whose weights AP equals the
  immediately preceding ldweights (PE weight registers persist across
  matmuls) — the tile pipeline emits one ldweights per matmul even for
  back-to-back matmuls sharing a stationary.

Layouts (host-prepared, DMA-friendly):
  xT   [8, 128, C]      x[idx].T split along d into 8 k-tiles (bf16)
  gw/uw[32, 128, 8, 128] gate/up ^T tiled: [h_tile][d_sub][k][h] (bf16)
  dw   [8, 128, 4096]   down^T tiled:     [dout_tile][h_sub][hi*128+dout] (bf16)
  cwF  [128, C]         combine weights, replicated across partitions (f32)
"""
import sys, os
for p in ("/opt/trn_rl_repo", os.path.join(os.path.dirname(os.path.abspath(__file__)))):
    if p not in sys.path:
        sys.path.insert(0, p)
import numpy as np
import ml_dtypes

BF16 = ml_dtypes.bfloat16
D_MODEL = 1024
D_INNER = 4096
N_EXPERTS = 8
TOP_K = 2
H_TILES = D_INNER // 128  # 32
K_TILES = D_MODEL // 128  # 8


CAPACITY = 2048  # capacity-factor-1.0: perfect expert balance on device;
                 # the few overflow pairs are combined on the host in f32.


def _capacity(max_n: int) -> int:
    return max(256, min(CAPACITY, ((max_n + 127) // 128) * 128))


def _p1_groups(C: int):
    """Token groups for phase 1: <=512 wide (one PSUM bank), 128-aligned.
    Full 512-wide groups plus one tail measured fastest on HW."""
    return [512] * (C // 512) + ([C % 512] if C % 512 else [])


def _dedupe_ldweights(nc) -> int:
    """Drop InstLdweights that reload the exact stationary already in the
    PE array (identical weights AP as the immediately preceding ldweights,
    with no intervening weight-modifying instruction, and no sync of its
    own). PE weight registers persist across matmuls, so the reload is
    pure overhead on the PE queue."""
    removed = 0
    for b in nc.m.functions[0].blocks:
        insts = b.instructions
        last_sig = None
        drop = []
        for idx, i in enumerate(insts):
            tn = type(i).__name__
            if tn == "InstLdweights":
                ap = i.ins[0]
                sig = (
                    getattr(ap, "memref", None), ap.offset, str(ap.ap),
                    i.perf_mode, i.is_transpose, i.tile_position, i.tile_size,
                )
                si = i.sync_info
                has_sync = si is not None and (
                    len(si.on_wait) > 0 or len(si.on_update) > 0)
                if sig == last_sig and not has_sync:
                    drop.append(idx)
                    continue
                last_sig = sig
            elif tn == "InstMatmult":
                continue  # weights unchanged; keep last_sig live
            elif tn in ("InstEventSemaphore", "InstDMACopy"):
                continue  # no effect on PE weight registers
            else:
                last_sig = None  # conservative across anything else
        for idx in reversed(drop):
            del insts[idx]
        removed += len(drop)
    return removed


def _build_nc(C: int, reps: int = 1, fast_start: bool = True, polish: bool = False, tail128: bool = False, dedupe: bool = True, p1wide: bool = False, sw: int = 3, wide: bool = False, phases: str = "12"):
    import concourse.bass as bass
    import concourse.mybir as mybir
    import concourse.tile as tile
    from concourse import bacc
    from contextlib import nullcontext

    f32 = mybir.dt.float32
    bf16 = mybir.dt.bfloat16
    Silu = mybir.ActivationFunctionType.Silu

    assert C % 128 == 0
    groups = _p1_groups(C)
    assert sum(groups) == C and all(g <= 512 for g in groups)
    sweeps = [groups[i:i + sw] for i in range(0, len(groups), sw)]
    # phase-2 token chunks: 512-wide PSUM banks, dout-tile stationary
    q_chunks = [512] * (C // 512) + ([C % 512] if C % 512 else [])

    nc = bacc.Bacc(None, target_bir_lowering=False)
    xT_d = nc.dram_tensor("xT", [K_TILES, 128, C], bf16, kind="ExternalInput")
    gw_d = nc.dram_tensor("gw", [H_TILES, 128, K_TILES, 128], bf16, kind="ExternalInput")
    uw_d = nc.dram_tensor("uw", [H_TILES, 128, K_TILES, 128], bf16, kind="ExternalInput")
    dw_d = nc.dram_tensor("dw", [K_TILES, 128, H_TILES * 128], bf16, kind="ExternalInput")
    cw_d = nc.dram_tensor("cwF", [128, C], f32, kind="ExternalInput")
    y_d = nc.dram_tensor("y", [D_MODEL, C], bf16, kind="ExternalOutput")
    hb_d = None
    if "1" not in phases:
        hb_d = nc.dram_tensor("hbin", [128, H_TILES, C], bf16, kind="ExternalInput")

    with tile.TileContext(nc) as tc:
        with (
            tc.tile_pool(name="xt", bufs=1) as xt_pool,
            tc.tile_pool(name="wgt", bufs=6 if polish else 4) as wgt_pool,
            tc.tile_pool(name="dwp", bufs=3) as dw_pool,
            tc.tile_pool(name="hb", bufs=1) as hb_pool,
            tc.tile_pool(name="sg", bufs=2 if p1wide else 3) as sg_pool,
            tc.tile_pool(name="yo", bufs=2 if p1wide else 4) as y_pool,
            tc.tile_pool(name="cw", bufs=1) as cw_pool,
            tc.tile_pool(name="ps", bufs=8, space="PSUM") as ps,
        ):
            cw_sb = cw_pool.tile([128, C], f32)
            nc.sync.dma_start(cw_sb[:], cw_d[:])

            rep_ctx = (
                tc.For_i(0, reps, 1,
                         hint_engines=(mybir.EngineType.PE, mybir.EngineType.SP))
                if reps > 1 else nullcontext()
            )
            with rep_ctx:
                hoisted = []
                if "1" not in phases:
                    fast_start = False
                    skip_x = True
                else:
                    skip_x = False
                if skip_x:
                    pass
                elif fast_start:
                    # hoist leading h-tiles' weight DMAs ahead of the bulk x
                    # transfer, and give each k-slice of x its own tile so
                    # the first matmuls gate on single-k DMAs
                    nhoist = 4 if polish else 1
                    for h in range(nhoist):
                        gwh = wgt_pool.tile([128, K_TILES, 128], bf16, tag="gw", name=f"gw{h}")
                        nc.sync.dma_start(gwh[:], gw_d[h])
                        uwh = wgt_pool.tile([128, K_TILES, 128], bf16, tag="uw", name=f"uw{h}")
                        nc.sync.dma_start(uwh[:], uw_d[h])
                        hoisted.append((gwh, uwh))
                    splitc = sum(sweeps[0]) if (polish and len(sweeps) > 1) else C
                    xtkA = [xt_pool.tile([128, splitc], bf16, tag=f"xtka{k}", name=f"xtka{k}")
                            for k in range(K_TILES)]
                    for k in range(K_TILES):
                        nc.sync.dma_start(xtkA[k][:], xT_d[k][:, 0:splitc])
                    xtkB = None
                    if splitc < C:
                        xtkB = [xt_pool.tile([128, C - splitc], bf16, tag=f"xtkb{k}", name=f"xtkb{k}")
                                for k in range(K_TILES)]
                        for k in range(K_TILES):
                            nc.sync.dma_start(xtkB[k][:], xT_d[k][:, splitc:])
                    def xs(k, gs, gsz):
                        if gs >= splitc:
                            return xtkB[k][:, gs - splitc:gs - splitc + gsz]
                        return xtkA[k][:, gs:gs + gsz]
                else:
                    xt = xt_pool.tile([128, K_TILES, C], bf16, tag="xt")
                    for k in range(K_TILES):
                        nc.sync.dma_start(xt[:, k, :], xT_d[k])
                    def xs(k, gs, gsz):
                        return xt[:, k, gs:gs + gsz]
                hbuf = hb_pool.tile([128, H_TILES, C], bf16, tag="hbuf")

                # ---- phase 1: gate/up + SwiGLU into hbuf
                if "1" not in phases:
                    # p2-only diagnostic: source hbuf from DRAM (32 DMAs,
                    # one per h-tile, overlap with phase-2 PE)
                    for hi in range(H_TILES):
                        nc.sync.dma_start(hbuf[:, hi, :], hb_d[:, hi, :])
                    sweeps_eff = []
                elif wide:
                    # one multi-bank (4x512) matmul per (hi, k). DO NOT USE:
                    # walrus rejects moving free dim >512
                    # (s3d3_mm_num_elements); kept only as a record.
                    assert not polish
                    gl = []
                    g0 = 0
                    for gsz in groups:
                        gl.append((g0, gsz))
                        g0 += gsz
                    for hi in range(H_TILES):
                        if hoisted:
                            gw, uw = hoisted.pop(0)
                        else:
                            gw = wgt_pool.tile([128, K_TILES, 128], bf16, tag="gw")
                            nc.sync.dma_start(gw[:], gw_d[hi])
                            uw = wgt_pool.tile([128, K_TILES, 128], bf16, tag="uw")
                            nc.sync.dma_start(uw[:], uw_d[hi])
                        pg = ps.tile([128, C], f32, tag="ps", name="pg")
                        for k in range(K_TILES):
                            nc.tensor.matmul(
                                pg[:], gw[:, k, :], xs(k, 0, C),
                                start=(k == 0), stop=(k == K_TILES - 1))
                        sgb = sg_pool.tile([128, C], bf16, tag="sg", name="sgb")
                        for (gs, gsz) in gl:
                            nc.scalar.activation(sgb[:, gs:gs + gsz], pg[:, gs:gs + gsz], Silu)
                        pu = ps.tile([128, C], f32, tag="ps", name="pu")
                        for k in range(K_TILES):
                            nc.tensor.matmul(
                                pu[:], uw[:, k, :], xs(k, 0, C),
                                start=(k == 0), stop=(k == K_TILES - 1))
                        for (gs, gsz) in gl:
                            nc.vector.tensor_mul(
                                hbuf[:, hi, gs:gs + gsz], sgb[:, gs:gs + gsz],
                                pu[:, gs:gs + gsz])
                    sweeps_eff = []
                elif p1wide:
                    # gate over ALL token groups per stationary (4 PSUM
                    # banks), silu into a bf16 staging row, then up over all
                    # groups (other 4 banks), mul into hbuf. Stationary runs
                    # of len(groups) amortize PE weight loads.
                    gl = []
                    g0 = 0
                    for gsz in groups:
                        gl.append((g0, gsz))
                        g0 += gsz
                    for hi in range(H_TILES):
                        if hoisted:
                            gw, uw = hoisted.pop(0)
                        else:
                            gw = wgt_pool.tile([128, K_TILES, 128], bf16, tag="gw")
                            nc.sync.dma_start(gw[:], gw_d[hi])
                            uw = wgt_pool.tile([128, K_TILES, 128], bf16, tag="uw")
                            nc.sync.dma_start(uw[:], uw_d[hi])
                        pg = [ps.tile([128, gsz], f32, tag="ps", name="pg",
                                      padded_shape=[128, 512]) for (_, gsz) in gl]
                        for k in range(K_TILES):
                            for j, (gs, gsz) in enumerate(gl):
                                nc.tensor.matmul(
                                    pg[j][:], gw[:, k, :], xs(k, gs, gsz),
                                    start=(k == 0), stop=(k == K_TILES - 1))
                        sgb = sg_pool.tile([128, C], bf16, tag="sg", name="sgb")
                        for j, (gs, gsz) in enumerate(gl):
                            nc.scalar.activation(sgb[:, gs:gs + gsz], pg[j][:], Silu)
                        pu = [ps.tile([128, gsz], f32, tag="ps", name="pu",
                                      padded_shape=[128, 512]) for (_, gsz) in gl]
                        for k in range(K_TILES):
                            for j, (gs, gsz) in enumerate(gl):
                                nc.tensor.matmul(
                                    pu[j][:], uw[:, k, :], xs(k, gs, gsz),
                                    start=(k == 0), stop=(k == K_TILES - 1))
                        for j, (gs, gsz) in enumerate(gl):
                            nc.vector.tensor_mul(
                                hbuf[:, hi, gs:gs + gsz], sgb[:, gs:gs + gsz], pu[j][:])
                    sweeps_eff = []
                else:
                    sweeps_eff = sweeps
                t0 = 0
                for sw in sweeps_eff:
                    sl = []
                    g0 = t0
                    for gsz in sw:
                        sl.append((g0, gsz))
                        g0 += gsz
                    for hi in range(H_TILES):
                        if hoisted:
                            gw, uw = hoisted.pop(0)
                        else:
                            gw = wgt_pool.tile([128, K_TILES, 128], bf16, tag="gw")
                            nc.sync.dma_start(gw[:], gw_d[hi])
                            uw = wgt_pool.tile([128, K_TILES, 128], bf16, tag="uw")
                            nc.sync.dma_start(uw[:], uw_d[hi])
                        pg = [ps.tile([128, gsz], f32, tag="ps", name="pg",
                                      padded_shape=[128, 512]) for (_, gsz) in sl]
                        pu = [ps.tile([128, gsz], f32, tag="ps", name="pu",
                                      padded_shape=[128, 512]) for (_, gsz) in sl]
                        for k in range(K_TILES):
                            for j, (gs, gsz) in enumerate(sl):
                                nc.tensor.matmul(
                                    pg[j][:], gw[:, k, :], xs(k, gs, gsz),
                                    start=(k == 0), stop=(k == K_TILES - 1))
                            for j, (gs, gsz) in enumerate(sl):
                                nc.tensor.matmul(
                                    pu[j][:], uw[:, k, :], xs(k, gs, gsz),
                                    start=(k == 0), stop=(k == K_TILES - 1))
                        for j, (gs, gsz) in enumerate(sl):
                            sg = sg_pool.tile([128, gsz], f32, tag="sg", name="sg",
                                              padded_shape=[128, 512])
                            nc.scalar.activation(sg[:], pg[j][:], Silu)
                            nc.vector.tensor_mul(hbuf[:, hi, gs:gs + gsz], sg[:], pu[j][:])
                    t0 = g0

                # ---- phase 2 (swapped): stationary = down_w tile
                # [128h, 128dout]; moving = hbuf token columns. Output is
                # y^T [dout, tokens]; scale by partition-replicated cw.
                HH = H_TILES // 2
                if "2" not in phases:
                    # p1-only diagnostic: still produce the output tensor
                    nc.sync.dma_start(y_d[0:128, :], hbuf[:, H_TILES - 1, :])
                for dt in (range(K_TILES) if "2" in phases else []):
                    dwt2 = []
                    for h2 in range(2):
                        dwh = dw_pool.tile([128, HH * 128], bf16, tag="dw", name="dwt")
                        nc.sync.dma_start(dwh[:], dw_d[dt][:, h2 * HH * 128:(h2 + 1) * HH * 128])
                        dwt2.append(dwh)
                    yp = [ps.tile([128, qs], f32, tag="ps", name="yp",
                                  padded_shape=[128, 512]) for qs in q_chunks]
                    for hi in range(H_TILES):
                        st = dwt2[hi // HH][:, (hi % HH) * 128:(hi % HH + 1) * 128]
                        q0 = 0
                        for q, qs in enumerate(q_chunks):
                            nc.tensor.matmul(
                                yp[q][:], st, hbuf[:, hi, q0:q0 + qs],
                                start=(hi == 0), stop=(hi == H_TILES - 1))
                            q0 += qs
                    q0 = 0
                    for q, qs in enumerate(q_chunks):
                        yt = y_pool.tile([128, qs], bf16, tag="yt", name="yt",
                                         padded_shape=[128, 512])
                        nc.vector.tensor_mul(yt[:], yp[q][:], cw_sb[:, q0:q0 + qs])
                        nc.sync.dma_start(
                            y_d[dt * 128:(dt + 1) * 128, q0:q0 + qs], yt[:])
                        q0 += qs
    nc.compile()
    if dedupe:
        n = _dedupe_ldweights(nc)
        import logging
        logging.getLogger(__name__).info(f"deduped {n} ldweights")
    import concourse.bass as _bass
    _bass.Bass.finalize(nc)
    return nc


_NC_CACHE: dict = {}


def _get_nc(C: int):
    if C not in _NC_CACHE:
        _NC_CACHE[C] = _build_nc(C)
    return _NC_CACHE[C]


def _route(x2d: np.ndarray, router_w: np.ndarray, router_b: np.ndarray):
    """fp64 router: returns (idx_per_expert, cw_per_expert) lists."""
    logits = x2d.astype(np.float64) @ router_w.astype(np.float64).T + router_b.astype(np.float64)
    m = logits.max(axis=-1, keepdims=True)
    p = np.exp(logits - m)
    p /= p.sum(axis=-1, keepdims=True)
    # top-2 (jax.lax.top_k picks largest; softmax is monotonic in logits)
    i1 = np.argmax(p, axis=-1)
    p_masked = p.copy()
    p_masked[np.arange(p.shape[0]), i1] = -1.0
    i2 = np.argmax(p_masked, axis=-1)
    p1 = p[np.arange(p.shape[0]), i1]
    p2 = p[np.arange(p.shape[0]), i2]
    denom = p1 + p2
    w1 = p1 / denom
    w2 = p2 / denom
    idxs, cws = [], []
    for e in range(N_EXPERTS):
        sel1 = np.nonzero(i1 == e)[0]
        sel2 = np.nonzero(i2 == e)[0]
        idx = np.concatenate([sel1, sel2])
        cw = np.concatenate([w1[sel1], w2[sel2]])
        idxs.append(idx)
        cws.append(cw.astype(np.float32))
    return idxs, cws


def _prep_core_inputs(x2d, idxs, cws, gate_w, up_w, down_w, C):
    in_maps = []
    for e in range(N_EXPERTS):
        idx = idxs[e]
        n = len(idx)
        xe = np.zeros((C, D_MODEL), np.float32)
        xe[:n] = x2d[idx]
        xT = np.ascontiguousarray(xe.T).astype(BF16).reshape(K_TILES, 128, C)
        gw = np.ascontiguousarray(
            gate_w[e].T.reshape(K_TILES, 128, H_TILES, 128).transpose(2, 1, 0, 3)).astype(BF16)
        uw = np.ascontiguousarray(
            up_w[e].T.reshape(K_TILES, 128, H_TILES, 128).transpose(2, 1, 0, 3)).astype(BF16)
        # [dout_tile, h_sub, hi*128+dout]: contiguous stationary tiles for
        # the swapped phase 2 (one 1MB DMA per dout tile)
        dw = np.ascontiguousarray(
            down_w[e].T.reshape(H_TILES, 128, K_TILES, 128)
            .transpose(2, 1, 0, 3).reshape(K_TILES, 128, H_TILES * 128)).astype(BF16)
        cw = np.zeros((C,), np.float32)
        cw[:n] = cws[e]
        cwF = np.ascontiguousarray(np.broadcast_to(cw[None, :], (128, C)))
        in_maps.append({"xT": xT, "gw": gw, "uw": uw, "dw": dw, "cwF": cwF})
    return in_maps


def _silu(v):
    return v / (1.0 + np.exp(-v))


def kernel(x, router_w, router_b, gate_w, up_w, down_w):
    from concourse.bass_utils import run_bass_kernel_spmd

    x = np.asarray(x, dtype=np.float32)
    router_w = np.asarray(router_w, dtype=np.float32)
    router_b = np.asarray(router_b, dtype=np.float32)
    gate_w = np.asarray(gate_w, dtype=np.float32)
    up_w = np.asarray(up_w, dtype=np.float32)
    down_w = np.asarray(down_w, dtype=np.float32)

    B, S, D = x.shape
    x2d = x.reshape(B * S, D)
    idxs, cws = _route(x2d, router_w, router_b)
    max_n = max(len(i) for i in idxs)
    C = _capacity(max_n)

    # device gets the first C pairs per expert; overflow handled on host
    dev_idxs = [i[:C] for i in idxs]
    dev_cws = [c[:C] for c in cws]

    nc = _get_nc(C)
    in_maps = _prep_core_inputs(x2d, dev_idxs, dev_cws, gate_w, up_w, down_w, C)
    res = run_bass_kernel_spmd(nc, in_maps, core_ids=list(range(N_EXPERTS)), trace=False)

    out = np.zeros((B * S, D_MODEL), np.float32)
    for e in range(N_EXPERTS):
        n = len(dev_idxs[e])
        ye = res.results[e]["y"].astype(np.float32).T  # [C, D_MODEL]
        np.add.at(out, dev_idxs[e], ye[:n])
        if len(idxs[e]) > C:  # capacity overflow: combine on host in f32
            oi = idxs[e][C:]
            ocw = cws[e][C:]
            xs = x2d[oi]
            h = _silu(xs @ gate_w[e].T) * (xs @ up_w[e].T)
            np.add.at(out, oi, ocw[:, None] * (h @ down_w[e].T))
    return out.reshape(B, S, D_MODEL)



# revision 26
# speedup vs baseline: 1.0283x; 1.0004x over previous
"""MoE FFN (8 experts, top-2) on 8 TRN2 NeuronCores, expert-parallel.

Strategy:
  - Host: router (fp64 logits -> softmax -> top-2 -> renormalized combine
    weights), gather each expert's assigned tokens, pad to a common
    capacity C (SPMD: one program, per-core inputs).
  - Core e: full SwiGLU FFN for expert e over its C tokens in bf16
    (full PE rate, half the DMA of fp32r), combine-weight scaling on
    device; outputs y^T [1024, C] bf16 (host transposes on unshard).
  - Host: scatter-add per-expert outputs back into [B, S, D].

Device kernel structure (single pass over all C tokens):
  Phase 1 (gate/up): x kept fully in SBUF; sweeps of <=3 token groups
    (512-col PSUM banks, 128-aligned widths), k-outer matmul ordering so
    one stationary (weight k-tile) feeds all groups of a sweep; SwiGLU
    into a resident bf16 hbuf [128,32,C].
    (Measured on HW: per-matmul instruction issue and ldweights reloads
    are ~free; >512-col matmuls are rejected by walrus
    (s3d3_mm_num_elements), so 512-col groups are already optimal.)
  Phase 2 (down): SWAPPED operands — stationary = down_w tile
    [128h, 128dout], moving = hbuf token columns (4 matmuls x 512 per
    stationary). Amortizes PE weight loads 4x and streams down_w once
    (not once per token chunk). Accumulate over 32 h-tiles into 4 PSUM
    banks per dout tile (2 dout tiles in flight = 8 banks). Output is
    y^T [dout, tokens]; combine-weight scale on evac via a
    partition-replicated cw row.

  Post-compile pass: delete InstLdweights whose weights AP equals the
  immediately preceding ldweights (PE weight registers persist across
  matmuls) — the tile pipeline emits one ldweights per matmul even for
  back-to-back matmuls sharing a stationary.

Layouts (host-prepared, DMA-friendly):
  xT   [8, 128, C]      x[idx].T split along d into 8 k-tiles (bf16)
  gw/uw[32, 128, 8, 128] gate/up ^T tiled: [h_tile][d_sub][k][h] (bf16)
  dw   [8, 128, 4096]   down^T tiled:     [dout_tile][h_sub][hi*128+dout] (bf16)
  cwF  [128, C]         combine weights, replicated across partitions (f32)
"""
import sys, os
for p in ("/opt/trn_rl_repo", os.path.join(os.path.dirname(os.path.abspath(__file__)))):
    if p not in sys.path:
        sys.path.insert(0, p)
import numpy as np
import ml_dtypes

BF16 = ml_dtypes.bfloat16
D_MODEL = 1024
D_INNER = 4096
N_EXPERTS = 8
TOP_K = 2
H_TILES = D_INNER // 128  # 32
K_TILES = D_MODEL // 128  # 8


CAPACITY = 2048  # capacity-factor-1.0: perfect expert balance on device;
                 # the few overflow pairs are combined on the host in f32.


def _capacity(max_n: int) -> int:
    return max(256, min(CAPACITY, ((max_n + 127) // 128) * 128))


def _p1_groups(C: int):
    """Token groups for phase 1: <=512 wide (one PSUM bank), 128-aligned.
    Full 512-wide groups plus one tail measured fastest on HW."""
    return [512] * (C // 512) + ([C % 512] if C % 512 else [])


def _dedupe_ldweights(nc) -> int:
    """Drop InstLdweights that reload the exact stationary already in the
    PE array (identical weights AP as the immediately preceding ldweights,
    with no intervening weight-modifying instruction, and no sync of its
    own). PE weight registers persist across matmuls, so the reload is
    pure overhead on the PE queue."""
    removed = 0
    for b in nc.m.functions[0].blocks:
        insts = b.instructions
        last_sig = None
        drop = []
        for idx, i in enumerate(insts):
            tn = type(i).__name__
            if tn == "InstLdweights":
                ap = i.ins[0]
                sig = (
                    getattr(ap, "memref", None), ap.offset, str(ap.ap),
                    i.perf_mode, i.is_transpose, i.tile_position, i.tile_size,
                )
                si = i.sync_info
                has_sync = si is not None and (
                    len(si.on_wait) > 0 or len(si.on_update) > 0)
                if sig == last_sig and not has_sync:
                    drop.append(idx)
                    continue
                last_sig = sig
            elif tn == "InstMatmult":
                continue  # weights unchanged; keep last_sig live
            elif tn in ("InstEventSemaphore", "InstDMACopy"):
                continue  # no effect on PE weight registers
            else:
                last_sig = None  # conservative across anything else
        for idx in reversed(drop):
            del insts[idx]
        removed += len(drop)
    return removed


def _build_nc(C: int, reps: int = 1, fast_start: bool = True, polish: bool = False, tail128: bool = False, dedupe: bool = True, p1wide: bool = False, sw: int = 3, wide: bool = False, phases: str = "12", wgtb: int = 4, dwpb: int = 3):
    import concourse.bass as bass
    import concourse.mybir as mybir
    import concourse.tile as tile
    from concourse import bacc
    from contextlib import nullcontext

    f32 = mybir.dt.float32
    bf16 = mybir.dt.bfloat16
    Silu = mybir.ActivationFunctionType.Silu

    assert C % 128 == 0
    groups = _p1_groups(C)
    assert sum(groups) == C and all(g <= 512 for g in groups)
    sweeps = [groups[i:i + sw] for i in range(0, len(groups), sw)]
    # phase-2 token chunks: 512-wide PSUM banks, dout-tile stationary
    q_chunks = [512] * (C // 512) + ([C % 512] if C % 512 else [])

    nc = bacc.Bacc(None, target_bir_lowering=False)
    xT_d = nc.dram_tensor("xT", [K_TILES, 128, C], bf16, kind="ExternalInput")
    gw_d = nc.dram_tensor("gw", [H_TILES, 128, K_TILES, 128], bf16, kind="ExternalInput")
    uw_d = nc.dram_tensor("uw", [H_TILES, 128, K_TILES, 128], bf16, kind="ExternalInput")
    dw_d = nc.dram_tensor("dw", [K_TILES, 128, H_TILES * 128], bf16, kind="ExternalInput")
    cw_d = nc.dram_tensor("cwF", [128, C], f32, kind="ExternalInput")
    y_d = nc.dram_tensor("y", [D_MODEL, C], bf16, kind="ExternalOutput")
    hb_d = None
    if "1" not in phases:
        hb_d = nc.dram_tensor("hbin", [128, H_TILES, C], bf16, kind="ExternalInput")

    with tile.TileContext(nc) as tc:
        with (
            tc.tile_pool(name="xt", bufs=1) as xt_pool,
            tc.tile_pool(name="wgt", bufs=6 if polish else wgtb) as wgt_pool,
            tc.tile_pool(name="dwp", bufs=dwpb) as dw_pool,
            tc.tile_pool(name="hb", bufs=1) as hb_pool,
            tc.tile_pool(name="sg", bufs=2 if p1wide else 3) as sg_pool,
            tc.tile_pool(name="yo", bufs=2 if p1wide else 4) as y_pool,
            tc.tile_pool(name="cw", bufs=1) as cw_pool,
            tc.tile_pool(name="ps", bufs=8, space="PSUM") as ps,
        ):
            cw_sb = cw_pool.tile([128, C], f32)
            nc.sync.dma_start(cw_sb[:], cw_d[:])

            rep_ctx = (
                tc.For_i(0, reps, 1,
                         hint_engines=(mybir.EngineType.PE, mybir.EngineType.SP))
                if reps > 1 else nullcontext()
            )
            with rep_ctx:
                hoisted = []
                if "1" not in phases:
                    fast_start = False
                    skip_x = True
                else:
                    skip_x = False
                if skip_x:
                    pass
                elif fast_start:
                    # hoist leading h-tiles' weight DMAs ahead of the bulk x
                    # transfer, and give each k-slice of x its own tile so
                    # the first matmuls gate on single-k DMAs
                    nhoist = 4 if polish else 1
                    for h in range(nhoist):
                        gwh = wgt_pool.tile([128, K_TILES, 128], bf16, tag="gw", name=f"gw{h}")
                        nc.sync.dma_start(gwh[:], gw_d[h])
                        uwh = wgt_pool.tile([128, K_TILES, 128], bf16, tag="uw", name=f"uw{h}")
                        nc.sync.dma_start(uwh[:], uw_d[h])
                        hoisted.append((gwh, uwh))
                    splitc = sum(sweeps[0]) if (polish and len(sweeps) > 1) else C
                    xtkA = [xt_pool.tile([128, splitc], bf16, tag=f"xtka{k}", name=f"xtka{k}")
                            for k in range(K_TILES)]
                    for k in range(K_TILES):
                        nc.sync.dma_start(xtkA[k][:], xT_d[k][:, 0:splitc])
                    xtkB = None
                    if splitc < C:
                        xtkB = [xt_pool.tile([128, C - splitc], bf16, tag=f"xtkb{k}", name=f"xtkb{k}")
                                for k in range(K_TILES)]
                        for k in range(K_TILES):
                            nc.sync.dma_start(xtkB[k][:], xT_d[k][:, splitc:])
                    def xs(k, gs, gsz):
                        if gs >= splitc:
                            return xtkB[k][:, gs - splitc:gs - splitc + gsz]
                        return xtkA[k][:, gs:gs + gsz]
                else:
                    xt = xt_pool.tile([128, K_TILES, C], bf16, tag="xt")
                    for k in range(K_TILES):
                        nc.sync.dma_start(xt[:, k, :], xT_d[k])
                    def xs(k, gs, gsz):
                        return xt[:, k, gs:gs + gsz]
                hbuf = hb_pool.tile([128, H_TILES, C], bf16, tag="hbuf")

                # ---- phase 1: gate/up + SwiGLU into hbuf
                if "1" not in phases:
                    # p2-only diagnostic: source hbuf from DRAM (32 DMAs,
                    # one per h-tile, overlap with phase-2 PE)
                    for hi in range(H_TILES):
                        nc.sync.dma_start(hbuf[:, hi, :], hb_d[:, hi, :])
                    sweeps_eff = []
                elif wide:
                    # one multi-bank (4x512) matmul per (hi, k). DO NOT USE:
                    # walrus rejects moving free dim >512
                    # (s3d3_mm_num_elements); kept only as a record.
                    assert not polish
                    gl = []
                    g0 = 0
                    for gsz in groups:
                        gl.append((g0, gsz))
                        g0 += gsz
                    for hi in range(H_TILES):
                        if hoisted:
                            gw, uw = hoisted.pop(0)
                        else:
                            gw = wgt_pool.tile([128, K_TILES, 128], bf16, tag="gw")
                            nc.sync.dma_start(gw[:], gw_d[hi])
                            uw = wgt_pool.tile([128, K_TILES, 128], bf16, tag="uw")
                            nc.sync.dma_start(uw[:], uw_d[hi])
                        pg = ps.tile([128, C], f32, tag="ps", name="pg")
                        for k in range(K_TILES):
                            nc.tensor.matmul(
                                pg[:], gw[:, k, :], xs(k, 0, C),
                                start=(k == 0), stop=(k == K_TILES - 1))
                        sgb = sg_pool.tile([128, C], bf16, tag="sg", name="sgb")
                        for (gs, gsz) in gl:
                            nc.scalar.activation(sgb[:, gs:gs + gsz], pg[:, gs:gs + gsz], Silu)
                        pu = ps.tile([128, C], f32, tag="ps", name="pu")
                        for k in range(K_TILES):
                            nc.tensor.matmul(
                                pu[:], uw[:, k, :], xs(k, 0, C),
                                start=(k == 0), stop=(k == K_TILES - 1))
                        for (gs, gsz) in gl:
                            nc.vector.tensor_mul(
                                hbuf[:, hi, gs:gs + gsz], sgb[:, gs:gs + gsz],
                                pu[:, gs:gs + gsz])
                    sweeps_eff = []
                elif p1wide:
                    # gate over ALL token groups per stationary (4 PSUM
                    # banks), silu into a bf16 staging row, then up over all
                    # groups (other 4 banks), mul into hbuf. Stationary runs
                    # of len(groups) amortize PE weight loads.
                    gl = []
                    g0 = 0
                    for gsz in groups:
                        gl.append((g0, gsz))
                        g0 += gsz
                    for hi in range(H_TILES):
                        if hoisted:
                            gw, uw = hoisted.pop(0)
                        else:
                            gw = wgt_pool.tile([128, K_TILES, 128], bf16, tag="gw")
                            nc.sync.dma_start(gw[:], gw_d[hi])
                            uw = wgt_pool.tile([128, K_TILES, 128], bf16, tag="uw")
                            nc.sync.dma_start(uw[:], uw_d[hi])
                        pg = [ps.tile([128, gsz], f32, tag="ps", name="pg",
                                      padded_shape=[128, 512]) for (_, gsz) in gl]
                        for k in range(K_TILES):
                            for j, (gs, gsz) in enumerate(gl):
                                nc.tensor.matmul(
                                    pg[j][:], gw[:, k, :], xs(k, gs, gsz),
                                    start=(k == 0), stop=(k == K_TILES - 1))
                        sgb = sg_pool.tile([128, C], bf16, tag="sg", name="sgb")
                        for j, (gs, gsz) in enumerate(gl):
                            nc.scalar.activation(sgb[:, gs:gs + gsz], pg[j][:], Silu)
                        pu = [ps.tile([128, gsz], f32, tag="ps", name="pu",
                                      padded_shape=[128, 512]) for (_, gsz) in gl]
                        for k in range(K_TILES):
                            for j, (gs, gsz) in enumerate(gl):
                                nc.tensor.matmul(
                                    pu[j][:], uw[:, k, :], xs(k, gs, gsz),
                                    start=(k == 0), stop=(k == K_TILES - 1))
                        for j, (gs, gsz) in enumerate(gl):
                            nc.vector.tensor_mul(
                                hbuf[:, hi, gs:gs + gsz], sgb[:, gs:gs + gsz], pu[j][:])
                    sweeps_eff = []
                else:
                    sweeps_eff = sweeps
                t0 = 0
                for sw in sweeps_eff:
                    sl = []
                    g0 = t0
                    for gsz in sw:
                        sl.append((g0, gsz))
                        g0 += gsz
                    for hi in range(H_TILES):
                        if hoisted:
                            gw, uw = hoisted.pop(0)
                        else:
                            gw = wgt_pool.tile([128, K_TILES, 128], bf16, tag="gw")
                            nc.sync.dma_start(gw[:], gw_d[hi])
                            uw = wgt_pool.tile([128, K_TILES, 128], bf16, tag="uw")
                            nc.sync.dma_start(uw[:], uw_d[hi])
                        pg = [ps.tile([128, gsz], f32, tag="ps", name="pg",
                                      padded_shape=[128, 512]) for (_, gsz) in sl]
                        pu = [ps.tile([128, gsz], f32, tag="ps", name="pu",
                                      padded_shape=[128, 512]) for (_, gsz) in sl]
                        for k in range(K_TILES):
                            for j, (gs, gsz) in enumerate(sl):
                                nc.tensor.matmul(
                                    pg[j][:], gw[:, k, :], xs(k, gs, gsz),
                                    start=(k == 0), stop=(k == K_TILES - 1))
                            for j, (gs, gsz) in enumerate(sl):
                                nc.tensor.matmul(
                                    pu[j][:], uw[:, k, :], xs(k, gs, gsz),
                                    start=(k == 0), stop=(k == K_TILES - 1))
                        for j, (gs, gsz) in enumerate(sl):
                            sg = sg_pool.tile([128, gsz], f32, tag="sg", name="sg",
                                              padded_shape=[128, 512])
                            nc.scalar.activation(sg[:], pg[j][:], Silu)
                            nc.vector.tensor_mul(hbuf[:, hi, gs:gs + gsz], sg[:], pu[j][:])
                    t0 = g0

                # ---- phase 2 (swapped): stationary = down_w tile
                # [128h, 128dout]; moving = hbuf token columns. Output is
                # y^T [dout, tokens]; scale by partition-replicated cw.
                HH = H_TILES // 2
                if "2" not in phases:
                    # p1-only diagnostic: still produce the output tensor
                    nc.sync.dma_start(y_d[0:128, :], hbuf[:, H_TILES - 1, :])
                for dt in (range(K_TILES) if "2" in phases else []):
                    dwt2 = []
                    for h2 in range(2):
                        dwh = dw_pool.tile([128, HH * 128], bf16, tag="dw", name="dwt")
                        nc.sync.dma_start(dwh[:], dw_d[dt][:, h2 * HH * 128:(h2 + 1) * HH * 128])
                        dwt2.append(dwh)
                    yp = [ps.tile([128, qs], f32, tag="ps", name="yp",
                                  padded_shape=[128, 512]) for qs in q_chunks]
                    for hi in range(H_TILES):
                        st = dwt2[hi // HH][:, (hi % HH) * 128:(hi % HH + 1) * 128]
                        q0 = 0
                        for q, qs in enumerate(q_chunks):
                            nc.tensor.matmul(
                                yp[q][:], st, hbuf[:, hi, q0:q0 + qs],
                                start=(hi == 0), stop=(hi == H_TILES - 1))
                            q0 += qs
                    q0 = 0
                    for q, qs in enumerate(q_chunks):
                        yt = y_pool.tile([128, qs], bf16, tag="yt", name="yt",
                                         padded_shape=[128, 512])
                        nc.vector.tensor_mul(yt[:], yp[q][:], cw_sb[:, q0:q0 + qs])
                        nc.sync.dma_start(
                            y_d[dt * 128:(dt + 1) * 128, q0:q0 + qs], yt[:])
                        q0 += qs
    nc.compile()
    if dedupe:
        n = _dedupe_ldweights(nc)
        import logging
        logging.getLogger(__name__).info(f"deduped {n} ldweights")
    import concourse.bass as _bass
    _bass.Bass.finalize(nc)
    return nc


_NC_CACHE: dict = {}


def _get_nc(C: int):
    if C not in _NC_CACHE:
        _NC_CACHE[C] = _build_nc(C)
    return _NC_CACHE[C]


def _route(x2d: np.ndarray, router_w: np.ndarray, router_b: np.ndarray):
    """fp64 router: returns (idx_per_expert, cw_per_expert) lists."""
    logits = x2d.astype(np.float64) @ router_w.astype(np.float64).T + router_b.astype(np.float64)
    m = logits.max(axis=-1, keepdims=True)
    p = np.exp(logits - m)
    p /= p.sum(axis=-1, keepdims=True)
    # top-2 (jax.lax.top_k picks largest; softmax is monotonic in logits)
    i1 = np.argmax(p, axis=-1)
    p_masked = p.copy()
    p_masked[np.arange(p.shape[0]), i1] = -1.0
    i2 = np.argmax(p_masked, axis=-1)
    p1 = p[np.arange(p.shape[0]), i1]
    p2 = p[np.arange(p.shape[0]), i2]
    denom = p1 + p2
    w1 = p1 / denom
    w2 = p2 / denom
    idxs, cws = [], []
    for e in range(N_EXPERTS):
        sel1 = np.nonzero(i1 == e)[0]
        sel2 = np.nonzero(i2 == e)[0]
        idx = np.concatenate([sel1, sel2])
        cw = np.concatenate([w1[sel1], w2[sel2]])
        idxs.append(idx)
        cws.append(cw.astype(np.float32))
    return idxs, cws


def _prep_core_inputs(x2d, idxs, cws, gate_w, up_w, down_w, C):
    in_maps = []
    for e in range(N_EXPERTS):
        idx = idxs[e]
        n = len(idx)
        xe = np.zeros((C, D_MODEL), np.float32)
        xe[:n] = x2d[idx]
        xT = np.ascontiguousarray(xe.T).astype(BF16).reshape(K_TILES, 128, C)
        gw = np.ascontiguousarray(
            gate_w[e].T.reshape(K_TILES, 128, H_TILES, 128).transpose(2, 1, 0, 3)).astype(BF16)
        uw = np.ascontiguousarray(
            up_w[e].T.reshape(K_TILES, 128, H_TILES, 128).transpose(2, 1, 0, 3)).astype(BF16)
        # [dout_tile, h_sub, hi*128+dout]: contiguous stationary tiles for
        # the swapped phase 2 (one 1MB DMA per dout tile)
        dw = np.ascontiguousarray(
            down_w[e].T.reshape(H_TILES, 128, K_TILES, 128)
            .transpose(2, 1, 0, 3).reshape(K_TILES, 128, H_TILES * 128)).astype(BF16)
        cw = np.zeros((C,), np.float32)
        cw[:n] = cws[e]
        cwF = np.ascontiguousarray(np.broadcast_to(cw[None, :], (128, C)))
        in_maps.append({"xT": xT, "gw": gw, "uw": uw, "dw": dw, "cwF": cwF})
    return in_maps


def _silu(v):
    return v / (1.0 + np.exp(-v))


def kernel(x, router_w, router_b, gate_w, up_w, down_w):
    from concourse.bass_utils import run_bass_kernel_spmd

    x = np.asarray(x, dtype=np.float32)
    router_w = np.asarray(router_w, dtype=np.float32)
    router_b = np.asarray(router_b, dtype=np.float32)
    gate_w = np.asarray(gate_w, dtype=np.float32)
    up_w = np.asarray(up_w, dtype=np.float32)
    down_w = np.asarray(down_w, dtype=np.float32)

    B, S, D = x.shape
    x2d = x.reshape(B * S, D)
    idxs, cws = _route(x2d, router_w, router_b)
    max_n = max(len(i) for i in idxs)
    C = _capacity(max_n)

    # device gets the first C pairs per expert; overflow handled on host
    dev_idxs = [i[:C] for i in idxs]
    dev_cws = [c[:C] for c in cws]

    nc = _get_nc(C)
    in_maps = _prep_core_inputs(x2d, dev_idxs, dev_cws, gate_w, up_w, down_w, C)
    res = run_bass_kernel_spmd(nc, in_maps, core_ids=list(range(N_EXPERTS)), trace=False)

    out = np.zeros((B * S, D_MODEL), np.float32)
    for e in range(N_EXPERTS):
        n = len(dev_idxs[e])
        ye = res.results[e]["y"].astype(np.float32).T  # [C, D_MODEL]
        np.add.at(out, dev_idxs[e], ye[:n])
        if len(idxs[e]) > C:  # capacity overflow: combine on host in f32
            oi = idxs[e][C:]
            ocw = cws[e][C:]
            xs = x2d[oi]
            h = _silu(xs @ gate_w[e].T) * (xs @ up_w[e].T)
            np.add.at(out, oi, ocw[:, None] * (h @ down_w[e].T))
    return out.reshape(B, S, D_MODEL)

